# revision 1
# baseline (speedup 1.0000x reference)
"""DEDICOM decoder forward on 8 Trainium2 NeuronCores.

Math per relation k (k=0..7):
    M_k = diag(d_k) @ G @ diag(d_k)                  (64x64, host-precomputed)
    out[k, n] = sigmoid( (row_n @ M_k) . col_n )

Device algorithm (data-parallel over N across 8 cores; per core SHARD=62500
samples padded to 489*128):
  - samples live 128-per-tile on SBUF partitions (partition p holds a
    contiguous HBM chunk so input DMA descriptors are fully contiguous)
  - PE: transpose row tile [128,64] -> [64,128] (bf16), then
        T = rowT.T @ M_all  with M_all = [64, 8*64] stacked M_k  -> PSUM fp32
  - ACT: cast T -> bf16 SBUF
  - DVE: U = T * broadcast(col) ; rec[:,k] = sum_j U[:,k,j]
  - ACT: sigmoid
  - DMA out [shard, 8] fp32; host reassembles/transposes to [8, N]
"""

import sys

sys.path.insert(0, "/opt/trn_rl_repo")

import numpy as np
import ml_dtypes

import concourse.bass as bass
import concourse.bacc as bacc
import concourse.mybir as mybir
from concourse import tile
from concourse.bass_utils import run_bass_kernel_spmd

N, D, R = 500000, 64, 8
NCORES = 8
SHARD = N // NCORES            # 62500
TPP = 490                      # samples per partition; 490*128 = 62720 >= 62500
SHARD_PAD = TPP * 128
W = 70                         # group width (samples/partition/group); 7 groups
NGROUPS = TPP // W
BF16 = mybir.dt.bfloat16
F32 = mybir.dt.float32

_CACHE: dict = {}


def _build_program():
    if "nc" in _CACHE:
        return _CACHE["nc"]

    nc = bacc.Bacc(
        "TRN2", target_bir_lowering=False, debug=False, num_devices=NCORES
    )

    row_d = nc.dram_tensor("row", [SHARD_PAD, D], F32, kind="ExternalInput")
    col_d = nc.dram_tensor("col", [SHARD_PAD, D], F32, kind="ExternalInput")
    mall_d = nc.dram_tensor("mall", [D, R * D], F32, kind="ExternalInput")
    ident_d = nc.dram_tensor("ident", [128, 128], BF16, kind="ExternalInput")
    out_d = nc.dram_tensor("out", [SHARD_PAD, R], F32, kind="ExternalOutput")

    row_v = row_d.ap().rearrange("(p t) d -> p t d", p=128)
    col_v = col_d.ap().rearrange("(p t) d -> p t d", p=128)
    out_v = out_d.ap().rearrange("(p t) k -> p t k", p=128)

    X = mybir.AxisListType.X
    ADD = mybir.AluOpType.add
    MULT = mybir.AluOpType.mult

    with tile.TileContext(nc) as tc:
        with (
            tc.tile_pool(name="const", bufs=1) as cpool,
            tc.tile_pool(name="io", bufs=2) as iopool,
            tc.tile_pool(name="work", bufs=4) as wpool,
            tc.tile_pool(name="psum_t", bufs=3, space="PSUM") as pt_pool,
            tc.tile_pool(name="psum_r", bufs=2, space="PSUM") as pr_pool,
        ):
            mall = cpool.tile([D, R * D], BF16, tag="mall")
            ident = cpool.tile([128, 128], BF16, tag="ident")
            nc.gpsimd.dma_start(mall[:], mall_d.ap())  # casts f32 -> bf16
            nc.sync.dma_start(ident[:], ident_d.ap())

            for g in range(NGROUPS):
                t0 = g * W
                row_g = iopool.tile([128, W, D], BF16, tag="row_g")
                col_g = iopool.tile([128, W, D], BF16, tag="col_g")
                nc.gpsimd.dma_start(row_g[:], row_v[:, t0 : t0 + W, :])
                nc.gpsimd.dma_start(col_g[:], col_v[:, t0 : t0 + W, :])

                rec_g = wpool.tile([128, W, R], F32, tag="rec")

                for b0 in range(0, W, 8):
                    bw = min(8, W - b0)
                    rowT_ps = pr_pool.tile([64, 8, 128], BF16, tag="rowT")
                    rowT_sb = wpool.tile([64, 8, 128], BF16, tag="rowT_sb")
                    for i in range(bw):
                        t = b0 + i
                        nc.tensor.transpose(
                            rowT_ps[:, i, :], row_g[:, t, :], ident[:]
                        )
                    nc.scalar.copy(rowT_sb[:, :bw, :], rowT_ps[:, :bw, :])
                    for i in range(bw):
                        t = b0 + i
                        T_ps = pt_pool.tile([128, R * D], F32, tag="T")
                        nc.tensor.matmul(T_ps[:], rowT_sb[:, i, :], mall[:])
                        T_sb = wpool.tile([128, R, D], BF16, tag="T_sb")
                        nc.scalar.copy(
                            T_sb[:].rearrange("p k j -> p (k j)"), T_ps[:]
                        )
                        U = wpool.tile([128, R, D], BF16, tag="U")
                        colb = (
                            col_g[:, t, :]
                            .unsqueeze(1)
                            .broadcast_to([128, R, D])
                        )
                        nc.vector.tensor_tensor(
                            out=U[:], in0=T_sb[:], in1=colb, op=MULT
                        )
                        # pairwise-fold the 64-wide reduction with TT adds
                        # (~2x faster/elem than TENSOR_REDUCE), then reduce 16
                        U2 = wpool.tile([128, R, 32], BF16, tag="U2")
                        nc.vector.tensor_tensor(
                            out=U2[:], in0=U[:, :, 0:32],
                            in1=U[:, :, 32:64], op=ADD,
                        )
                        U3 = wpool.tile([128, R, 16], BF16, tag="U3")
                        nc.vector.tensor_tensor(
                            out=U3[:], in0=U2[:, :, 0:16],
                            in1=U2[:, :, 16:32], op=ADD,
                        )
                        nc.vector.tensor_reduce(
                            rec_g[:, t, :], U3[:], axis=X, op=ADD
                        )

                sig_g = wpool.tile([128, W, R], F32, tag="sig")
                nc.scalar.activation(
                    sig_g[:],
                    rec_g[:],
                    mybir.ActivationFunctionType.Sigmoid,
                )
                nc.sync.dma_start(out_v[:, t0 : t0 + W, :], sig_g[:])

    nc.compile()
    _CACHE["nc"] = nc
    return nc


def _prep_inputs(inputs_row, inputs_col, global_interaction, local_variation):
    d = np.asarray(local_variation, np.float32)
    g = np.asarray(global_interaction, np.float32)
    # M_all[i, (k, j)] = d[k, i] * G[i, j] * d[k, j]
    mall = np.einsum("ki,ij,kj->ikj", d, g, d).reshape(D, R * D)
    mall = np.ascontiguousarray(mall, np.float32)
    ident = np.eye(128, dtype=ml_dtypes.bfloat16)

    pad = SHARD_PAD - SHARD
    in_maps = []
    for c in range(NCORES):
        sl = slice(c * SHARD, (c + 1) * SHARD)
        rr = np.concatenate(
            [np.asarray(inputs_row[sl], np.float32), np.zeros((pad, D), np.float32)]
        )
        cc = np.concatenate(
            [np.asarray(inputs_col[sl], np.float32), np.zeros((pad, D), np.float32)]
        )
        in_maps.append(
            {
                "row": np.ascontiguousarray(rr),
                "col": np.ascontiguousarray(cc),
                "mall": mall,
                "ident": ident,
            }
        )
    return in_maps


def kernel(inputs_row, inputs_col, global_interaction, local_variation):
    nc = _build_program()
    in_maps = _prep_inputs(
        inputs_row, inputs_col, global_interaction, local_variation
    )
    res = run_bass_kernel_spmd(nc, in_maps, list(range(NCORES)))
    outs = [res.results[c]["out"][:SHARD] for c in range(NCORES)]
    full = np.concatenate(outs, axis=0)  # [N, 8] f32
    return np.ascontiguousarray(full.T)  # [8, N]


if __name__ == "__main__":
    rng = np.random.default_rng(0)
    inputs = {
        "inputs_row": rng.standard_normal((N, D), dtype=np.float32),
        "inputs_col": rng.standard_normal((N, D), dtype=np.float32),
        "global_interaction": rng.uniform(-0.2, 0.2, (D, D)).astype(np.float32),
        "local_variation": rng.uniform(-0.3, 0.3, (R, D)).astype(np.float32),
    }
    out = kernel(**inputs)
    print("out", out.shape, out.dtype, out[:, :3])



# revision 4
# speedup vs baseline: 1.0580x; 1.0580x over previous
"""DEDICOM decoder forward on 8 Trainium2 NeuronCores.

Math per relation k (k=0..7):
    M_k = diag(d_k) @ G @ diag(d_k)                  (64x64, host-precomputed)
    out[k, n] = sigmoid( (row_n @ M_k) . col_n )

v3 pipeline (data-parallel over N across 8 cores; per core 62500 samples
padded to 490 tiles x 128 consecutive samples):
  - row shipped to HBM as bf16 [SHARD_PAD, 128] (features zero-padded
    64->128 so the DMA xbar transpose is legal); per group one
    dma_start_transpose yields rowT [128, W*128] in SBUF (features on
    partitions 0:64) -- no PE transposes, no PSUM round trip.
  - col shipped as bf16 [SHARD_PAD, 64]; loaded sample-major
    (partition p = sample 128t+p).
  - PE: per tile T = rowT_tile.T @ M_all -> PSUM f32 [128, 512],
    4 tiles per PSUM batch (4 banks), double buffered (8 banks).
  - ACT: batched copy PSUM f32 -> SBUF bf16.
  - DVE: U = T * broadcast(col) (2x mode), fold 64->32->16.
  - GPSIMD: folds 16->8->4->2->1 (f32 out).
  - ACT: sigmoid per group; DMA out [SHARD_PAD, 8] f32.
"""

import sys

sys.path.insert(0, "/opt/trn_rl_repo")

import numpy as np
import ml_dtypes

import concourse.bass as bass
import concourse.bacc as bacc
import concourse.mybir as mybir
from concourse import tile
from concourse.bass_utils import run_bass_kernel_spmd

N, D, R = 500000, 64, 8
NCORES = 8
SHARD = N // NCORES            # 62500
NTILES = 490                   # tiles of 128 consecutive samples
SHARD_PAD = NTILES * 128       # 62720
W = 70                         # tiles per group; 7 groups
NGROUPS = NTILES // W
B = 4                          # tiles per PSUM batch
BF16 = mybir.dt.bfloat16
F32 = mybir.dt.float32

_CACHE: dict = {}


def _build_program():
    if "nc" in _CACHE:
        return _CACHE["nc"]

    nc = bacc.Bacc(
        "TRN2", target_bir_lowering=False, debug=False, num_devices=NCORES
    )

    row_d = nc.dram_tensor("row", [SHARD_PAD, 2 * D], BF16, kind="ExternalInput")
    col_d = nc.dram_tensor("col", [SHARD_PAD, D], BF16, kind="ExternalInput")
    mall_d = nc.dram_tensor("mall", [D, R * D], BF16, kind="ExternalInput")
    out_d = nc.dram_tensor("out", [SHARD_PAD, R], F32, kind="ExternalOutput")

    row_v = row_d.ap()                                     # [SHARD_PAD, 128]
    col_v = col_d.ap().rearrange("(t p) d -> p t d", p=128)  # [128, 490, 64]
    out_v = out_d.ap().rearrange("(t p) k -> p t k", p=128)  # [128, 490, 8]

    ADD = mybir.AluOpType.add
    MULT = mybir.AluOpType.mult
    GS = W * 128                                            # samples per group

    with tile.TileContext(nc) as tc:
        with (
            tc.tile_pool(name="const", bufs=1) as cpool,
            tc.tile_pool(name="rowt", bufs=2) as rpool,
            tc.tile_pool(name="io", bufs=2) as iopool,
            tc.tile_pool(name="psum_t", bufs=2, space="PSUM") as pt_pool,
            tc.tile_pool(name="tsb", bufs=2) as tpool,
            tc.tile_pool(name="u", bufs=2) as upool,
            tc.tile_pool(name="fold", bufs=2) as fpool,
            tc.tile_pool(name="out", bufs=2) as opool,
        ):
            mall = cpool.tile([D, R * D], BF16, tag="mall")
            nc.sync.dma_start(mall[:], mall_d.ap())

            for g in range(NGROUPS):
                t0 = g * W
                rowT = rpool.tile([128, GS], BF16, tag="rowT")
                nc.sync.dma_start_transpose(
                    rowT[:], row_v[t0 * 128 : t0 * 128 + GS, :]
                )
                col_g = iopool.tile([128, W, D], BF16, tag="col_g")
                nc.gpsimd.dma_start(col_g[:], col_v[:, t0 : t0 + W, :])

                U3 = fpool.tile([128, W, R, 16], BF16, tag="U3")

                for q0 in range(0, W, B):
                    bw = min(B, W - q0)
                    T_ps = pt_pool.tile([128, B, R * D], F32, tag="T")
                    for i in range(bw):
                        t = q0 + i
                        nc.tensor.matmul(
                            T_ps[:, i, :],
                            rowT[0:64, (t * 128) : (t * 128 + 128)],
                            mall[:],
                        )
                    T_sb = tpool.tile([128, B, R, D], BF16, tag="T_sb")
                    nc.scalar.copy(
                        T_sb[:, :bw, :, :].rearrange("p b k j -> p b (k j)"),
                        T_ps[:, :bw, :],
                    )
                    U = upool.tile([128, B, R, D], BF16, tag="U")
                    colb = (
                        col_g[:, q0 : q0 + bw, :]
                        .unsqueeze(2)
                        .broadcast_to([128, bw, R, D])
                    )
                    nc.vector.tensor_tensor(
                        out=U[:, :bw], in0=T_sb[:, :bw], in1=colb, op=MULT
                    )
                    U2 = upool.tile([128, B, R, 32], BF16, tag="U2")
                    nc.vector.tensor_tensor(
                        out=U2[:, :bw],
                        in0=U[:, :bw, :, 0:32],
                        in1=U[:, :bw, :, 32:64],
                        op=ADD,
                    )
                    nc.vector.tensor_tensor(
                        out=U3[:, q0 : q0 + bw],
                        in0=U2[:, :bw, :, 0:16],
                        in1=U2[:, :bw, :, 16:32],
                        op=ADD,
                    )

                U4 = fpool.tile([128, W, R, 8], BF16, tag="U4")
                nc.gpsimd.tensor_tensor(
                    out=U4[:], in0=U3[:, :, :, 0:8], in1=U3[:, :, :, 8:16], op=ADD
                )
                U5 = fpool.tile([128, W, R, 4], BF16, tag="U5")
                nc.gpsimd.tensor_tensor(
                    out=U5[:], in0=U4[:, :, :, 0:4], in1=U4[:, :, :, 4:8], op=ADD
                )
                U6 = fpool.tile([128, W, R, 2], BF16, tag="U6")
                nc.gpsimd.tensor_tensor(
                    out=U6[:], in0=U5[:, :, :, 0:2], in1=U5[:, :, :, 2:4], op=ADD
                )
                rec = opool.tile([128, W, R], F32, tag="rec")
                nc.gpsimd.tensor_tensor(
                    out=rec[:].unsqueeze(3),
                    in0=U6[:, :, :, 0:1],
                    in1=U6[:, :, :, 1:2],
                    op=ADD,
                )

                sig = opool.tile([128, W, R], F32, tag="sig")
                nc.scalar.activation(
                    sig[:], rec[:], mybir.ActivationFunctionType.Sigmoid
                )
                nc.sync.dma_start(out_v[:, t0 : t0 + W, :], sig[:])

    nc.compile()
    _CACHE["nc"] = nc
    return nc


def _prep_inputs(inputs_row, inputs_col, global_interaction, local_variation):
    d = np.asarray(local_variation, np.float32)
    g = np.asarray(global_interaction, np.float32)
    # mall[i, (k, j)] = d[k, i] * G[i, j] * d[k, j]
    mall = np.einsum("ki,ij,kj->ikj", d, g, d).reshape(D, R * D)
    mall = np.ascontiguousarray(mall).astype(ml_dtypes.bfloat16)

    row16 = np.zeros((N, 2 * D), dtype=ml_dtypes.bfloat16)
    row16[:, :D] = np.asarray(inputs_row, np.float32)
    col16 = np.asarray(inputs_col, np.float32).astype(ml_dtypes.bfloat16)

    pad = SHARD_PAD - SHARD
    in_maps = []
    for c in range(NCORES):
        sl = slice(c * SHARD, (c + 1) * SHARD)
        rr = np.concatenate(
            [row16[sl], np.zeros((pad, 2 * D), ml_dtypes.bfloat16)]
        )
        cc = np.concatenate(
            [col16[sl], np.zeros((pad, D), ml_dtypes.bfloat16)]
        )
        in_maps.append(
            {
                "row": np.ascontiguousarray(rr),
                "col": np.ascontiguousarray(cc),
                "mall": mall,
            }
        )
    return in_maps


def kernel(inputs_row, inputs_col, global_interaction, local_variation):
    nc = _build_program()
    in_maps = _prep_inputs(
        inputs_row, inputs_col, global_interaction, local_variation
    )
    res = run_bass_kernel_spmd(nc, in_maps, list(range(NCORES)))
    outs = [res.results[c]["out"][:SHARD] for c in range(NCORES)]
    full = np.concatenate(outs, axis=0)  # [N, 8] f32
    return np.ascontiguousarray(full.T)  # [8, N]


if __name__ == "__main__":
    rng = np.random.default_rng(0)
    inputs = {
        "inputs_row": rng.standard_normal((N, D), dtype=np.float32),
        "inputs_col": rng.standard_normal((N, D), dtype=np.float32),
        "global_interaction": rng.uniform(-0.2, 0.2, (D, D)).astype(np.float32),
        "local_variation": rng.uniform(-0.3, 0.3, (R, D)).astype(np.float32),
    }
    out = kernel(**inputs)
    print("out", out.shape, out.dtype, out[:, :3])
